# revision 1
# baseline (speedup 1.0000x reference)
"""GQA (grouped-query attention) Trainium2 kernel, tensor-parallel over 8 NeuronCores.

Sharding: core c computes query heads {2c, 2c+1} and kv head c//2 (groups kept
intact), with wo row-sharded; each core returns a partial [B*S, HID] output and
the host sums the 8 partials.

Per-core device kernel (all matmuls in float32r: 1 cycle/row at N>=256,
~1.5e-4 rel err):
  phase 1: Q^T/K^T/V^T projections from host-transposed x^T, RoPE on Q/K (DVE),
           V transposed to natural layout via PE.
  phase 2: per (batch, 512-wide q-block, head): scores^T = K_blk Q^T (PE) ->
           exp (ACT, key_weight*scale folded into the activation scale) ->
           causal mask on diagonal blocks (DVE) -> attn^T accum + sum-of-exp
           via ones-matmul (PE). Softmax normalization is folded in as
           attn^T * broadcast(1/sum) where the broadcast is a K=1 PE matmul.
           wo projection consumes attn^T directly; output DMAd from PSUM.
"""
import numpy as np

B, S, HID = 2, 2048, 2048
NH, NKV, D = 16, 4, 128
NCORES = 8
HPC = NH // NCORES            # q heads per core
SQ = B * S                    # 4096 tokens
NKO = HID // 128              # 16 contraction chunks
NSQB = SQ // 512              # 8 sequence blocks of 512
JPB = S // 512                # 4 q-blocks per batch
KPB = S // 128                # 16 k-blocks per batch
ROPE_BASE = 10000.0
SCALE = float(D) ** -0.5

_cache = {}


def _consts():
    half = D // 2
    pos = np.arange(S, dtype=np.float32)
    inv_freq = (1.0 / (ROPE_BASE ** (np.arange(half, dtype=np.float32) / np.float32(half)))).astype(np.float32)
    ang = pos[:, None] * inv_freq[None, :]              # [S, 64]
    cos = np.cos(ang).astype(np.float32).T              # [64, S]
    sin = np.sin(ang).astype(np.float32).T
    cos_full = np.tile(np.concatenate([cos, cos], 0), (1, B))     # [128, SQ]
    sinpm = np.tile(np.concatenate([-sin, sin], 0), (1, B))       # [128, SQ]
    r = np.arange(128)
    tri = (r[:, None] <= r[None, :]).astype(np.float32)           # [128,128]
    zt = np.zeros((128, 256), np.float32)                         # [0 | tri]
    zt[:, 128:] = tri
    ones_col = np.ones((128, 1), np.float32)
    ones_row = np.ones((33, 128), np.float32)  # rows 0 and 32 used (per-head base partition)
    return cos_full, sinpm, tri, zt, ones_col, ones_row


def _build(phases=(1, 2)):
    import concourse.mybir as mybir
    from concourse import bacc
    from concourse.tile import TileContext

    f32 = mybir.dt.float32
    f32r = mybir.dt.float32r
    MUL = mybir.AluOpType.mult
    EXP = mybir.ActivationFunctionType.Exp
    CPY = mybir.ActivationFunctionType.Copy

    cos_np, sinpm_np, tri_np, zt_np, onescol_np, onesrow_np = _consts()

    nc = bacc.Bacc("TRN2", target_bir_lowering=False, debug=False)

    xT = nc.dram_tensor("xT", [HID, SQ], f32, kind="ExternalInput")
    wqT = nc.dram_tensor("wqT", [HID, HPC * D], f32, kind="ExternalInput")
    wkT = nc.dram_tensor("wkT", [HID, D], f32, kind="ExternalInput")
    wvT = nc.dram_tensor("wvT", [HID, D], f32, kind="ExternalInput")
    woT = nc.dram_tensor("woT", [HPC * D, HID], f32, kind="ExternalInput")
    kw = nc.dram_tensor("kw", [HPC], f32, kind="ExternalInput")
    out = nc.dram_tensor("out", [SQ, HID], f32, kind="ExternalOutput")

    cos_d = nc.inline_tensor(cos_np, name="cos_t")
    sinpm_d = nc.inline_tensor(sinpm_np, name="sinpm_t")
    tri_d = nc.inline_tensor(tri_np, name="tri_t")
    zt_d = nc.inline_tensor(zt_np, name="zt_t")
    onescol_d = nc.inline_tensor(onescol_np, name="onescol_t")
    onesrow_d = nc.inline_tensor(onesrow_np, name="onesrow_t")
    ident_d = nc.inline_tensor(np.eye(128, dtype=np.float32), name="ident_t")

    with TileContext(nc) as tc:
        with tc.tile_pool(name="persist", bufs=1) as pp:
            # persistent SBUF tensors
            wq_sb = pp.tile([128, NKO, HPC * D], f32r, tag="wq")
            wk_sb = pp.tile([128, NKO, D], f32r, tag="wk")
            wv_sb = pp.tile([128, NKO, D], f32r, tag="wv")
            wo_sb = pp.tile([128, HPC, HID], f32r, tag="wo")
            q_sb = [pp.tile([128, SQ], f32r, tag=f"q{h}", name=f"q_sb{h}") for h in range(HPC)]
            k_sb = pp.tile([128, SQ], f32r, tag="k")
            v_sb = pp.tile([128, SQ // 128, D], f32r, tag="v")
            cos_sb = pp.tile([128, SQ], f32, tag="cos")
            sinpm_sb = pp.tile([128, SQ], f32, tag="sinpm")
            tri_sb = pp.tile([128, 128], f32r, tag="tri")
            zt_sb = pp.tile([128, 256], f32r, tag="zt")
            onescol_sb = pp.tile([128, 1], f32r, tag="onescol")
            onesrow_sb = pp.tile([33, 128], f32r, tag="onesrow")
            kwsc_sb = pp.tile([128, HPC], f32, tag="kwsc")

            nc.sync.dma_start(wq_sb[:], wqT[:].rearrange("(ko p) d -> p ko d", p=128).bitcast(f32r))
            nc.sync.dma_start(wk_sb[:], wkT[:].rearrange("(ko p) d -> p ko d", p=128).bitcast(f32r))
            nc.sync.dma_start(wv_sb[:], wvT[:].rearrange("(ko p) d -> p ko d", p=128).bitcast(f32r))
            nc.sync.dma_start(wo_sb[:], woT[:].rearrange("(h p) e -> p h e", p=128).bitcast(f32r))
            nc.sync.dma_start(cos_sb[:], cos_d[:])
            nc.sync.dma_start(sinpm_sb[:], sinpm_d[:])
            nc.sync.dma_start(tri_sb[:], tri_d[:].bitcast(f32r))
            nc.sync.dma_start(zt_sb[:], zt_d[:].bitcast(f32r))
            nc.sync.dma_start(onescol_sb[:], onescol_d[:].bitcast(f32r))
            nc.sync.dma_start(onesrow_sb[:], onesrow_d[:].bitcast(f32r))
            kwraw_sb = pp.tile([128, HPC], f32, tag="kwraw")
            nc.sync.dma_start(kwraw_sb[:], kw[None, :].to_broadcast((128, HPC)))
            nc.vector.tensor_scalar_mul(kwsc_sb[:], kwraw_sb[:], SCALE)

            # ---------------- phase 1: projections + rope + V transpose --------
            ident_sb = pp.tile([128, 128], f32, tag="ident")
            nc.sync.dma_start(ident_sb[:], ident_d[:])

            from contextlib import ExitStack
            with ExitStack() as es:
                rt = es.enter_context(tc.tile_pool(name="rt", bufs=2))
                vs = es.enter_context(tc.tile_pool(name="vs", bufs=2))
                ep = es.enter_context(tc.tile_pool(name="ep", bufs=3))
                ap = es.enter_context(tc.tile_pool(name="ap", bufs=4))
                bp = es.enter_context(tc.tile_pool(name="bp", bufs=2))
                fsb = es.enter_context(tc.tile_pool(name="fsb", bufs=3))
                smp = es.enter_context(tc.tile_pool(name="smp", bufs=2))

                def rope(dst, src_ps, s0):
                    t0 = rt.tile([128, 512], f32, tag="t0")
                    t1 = rt.tile([128, 512], f32, tag="t1")
                    nc.vector.tensor_tensor(t0[:], src_ps[:], cos_sb[:, s0:s0 + 512], MUL)
                    nc.vector.tensor_tensor(t1[0:64, :], src_ps[64:128, :], sinpm_sb[0:64, s0:s0 + 512], MUL)
                    nc.vector.tensor_tensor(t1[64:128, :], src_ps[0:64, :], sinpm_sb[64:128, s0:s0 + 512], MUL)
                    nc.vector.tensor_tensor(dst, t0[:], t1[:], mybir.AluOpType.add)

                def proj_block(pps, tps, xp, sb):
                    s0 = sb * 512
                    qps = [pps.tile([128, 512], f32, tag=f"qps{h}", name=f"qps{h}") for h in range(HPC)]
                    kps = pps.tile([128, 512], f32, tag="kps", name="kps")
                    vps = pps.tile([128, 512], f32, tag="vps", name="vps")
                    for ko in range(NKO):
                        xt = xp.tile([128, 512], f32r, tag="x")
                        nc.sync.dma_start(xt[:], xT[ko * 128:(ko + 1) * 128, s0:s0 + 512].bitcast(f32r))
                        st, sp = (ko == 0), (ko == NKO - 1)
                        for h in range(HPC):
                            nc.tensor.matmul(qps[h][:], wq_sb[:, ko, h * D:(h + 1) * D], xt[:], start=st, stop=sp)
                        nc.tensor.matmul(kps[:], wk_sb[:, ko, :], xt[:], start=st, stop=sp)
                        nc.tensor.matmul(vps[:], wv_sb[:, ko, :], xt[:], start=st, stop=sp)
                    # V staging copy first (ACT) so the vps bank frees fast
                    vst = vs.tile([128, 512], f32, tag="vst")
                    nc.scalar.activation(vst[:], vps[:], CPY)
                    for h in range(HPC):
                        rope(q_sb[h][:, s0:s0 + 512], qps[h], s0)
                    rope(k_sb[:, s0:s0 + 512], kps, s0)
                    # V: transpose [d, s] -> natural [s, d]
                    for i in range(4):
                        vtp = tps.tile([128, 128], f32, tag="vt")
                        nc.tensor.transpose(vtp[:], vst[:, i * 128:(i + 1) * 128], ident_sb[:])
                        nc.scalar.activation(v_sb[:, sb * 4 + i, :], vtp[:], CPY)

                if 1 not in phases:  # ablation-timing only: fabricate phase-1 outputs
                    for h in range(HPC):
                        nc.gpsimd.memset(q_sb[h][:], 0.5)
                    nc.gpsimd.memset(k_sb[:], 0.5)
                    nc.gpsimd.memset(v_sb[:], 0.5)

                OFF = [0, 128, 256, 256]
                if 1 in phases:
                    with tc.tile_pool(name="pps", bufs=2, space="PSUM") as pps, \
                         tc.tile_pool(name="vps_p", bufs=1, space="PSUM") as vpsp, \
                         tc.tile_pool(name="tps", bufs=1, space="PSUM") as tps, \
                         tc.tile_pool(name="xp", bufs=4) as xp:
                        class _PP:  # qps*/kps from double-buffered pool, vps single
                            def tile(self, shape, dt_, tag, name):
                                return (vpsp if tag == "vps" else pps).tile(shape, dt_, tag=tag, name=name)
                        for sb in range(NSQB):
                            proj_block(_PP(), tps, xp, sb)
                p2 = ExitStack()
                if 2 in phases:
                    scps = p2.enter_context(tc.tile_pool(name="scps", bufs=2, space="PSUM"))
                    avps = p2.enter_context(tc.tile_pool(name="avps", bufs=2, space="PSUM"))
                    sups = p2.enter_context(tc.tile_pool(name="sups", bufs=1, space="PSUM"))
                    mfin = p2.enter_context(tc.tile_pool(name="mfin", bufs=3, space="PSUM"))
                for b in (range(B) if 2 in phases else ()):
                    t0 = b * S
                    for J in range(JPB):
                        q0 = t0 + J * 512
                        av_saved = []
                        # head h's sums live at partition 32*h (matmul operands
                        # must have base_partition in {0, 32, 64})
                        sums_bj = smp.tile([33, 512], f32, tag="sums")
                        for h in range(HPC):
                            avp = avps.tile([128, 512], f32, tag="av", name="avp")
                            sup = sups.tile([1, 512], f32, tag="su")
                            nkb = 4 * J + 4
                            for jj in range(nkb):
                                p = jj - 4 * J
                                off = OFF[p] if p >= 0 else 0
                                n = 512 - off
                                scp = scps.tile([128, 512], f32, tag="sc", name="scp")
                                nc.tensor.matmul(
                                    scp[:, 0:n],
                                    k_sb[:, t0 + jj * 128:t0 + (jj + 1) * 128],
                                    q_sb[h][:, q0 + off:q0 + 512],
                                    start=True, stop=True)
                                ex = ep.tile([128, 512], f32r, tag="ex")
                                nc.scalar.activation(ex[:, 0:n], scp[:, 0:n], EXP,
                                                     scale=kwsc_sb[:, h:h + 1])
                                if p == 3:
                                    # keep where col - 128 - row >= 0, else 0
                                    nc.gpsimd.affine_select(
                                        ex[:, 0:256], ex[:, 0:256], pattern=[[1, 256]],
                                        compare_op=mybir.AluOpType.is_ge, fill=0.0,
                                        base=-128, channel_multiplier=-1)
                                elif p >= 0:
                                    # keep where col - row >= 0 (causal diag block)
                                    nc.gpsimd.affine_select(
                                        ex[:, 0:128], ex[:, 0:128], pattern=[[1, 128]],
                                        compare_op=mybir.AluOpType.is_ge, fill=0.0,
                                        base=0, channel_multiplier=-1)
                                st, sp = (jj == 0), (jj == nkb - 1)
                                nc.tensor.matmul(avp[:, off:512], v_sb[:, b * KPB + jj, :],
                                                 ex[:, 0:n], start=st, stop=sp)
                                nc.tensor.matmul(sup[:, off:512], onescol_sb[:],
                                                 ex[:, 0:n], start=st, stop=sp)
                            nc.vector.tensor_copy(sums_bj[32 * h:32 * h + 1, :], sup[:])
                            av_saved.append(avp)
                        recip = smp.tile([33, 512], f32r, tag="recip")
                        with nc.allow_low_precision(reason="f32r rounding of softmax denom, ~1e-4 rel"):
                            nc.vector.reciprocal(recip[:], sums_bj[:])
                        attn_saved = []
                        for h in range(HPC):
                            bcp = mfin.tile([128, 512], f32, tag="mf", name="bcp")
                            nc.tensor.matmul(bcp[:], onesrow_sb[32 * h:32 * h + 1, :],
                                             recip[32 * h:32 * h + 1, :],
                                             start=True, stop=True)
                            bcs = bp.tile([128, 512], f32, tag="bcs")
                            nc.scalar.activation(bcs[:], bcp[:], CPY)
                            at = ap.tile([128, 512], f32r, tag="at")
                            nc.vector.tensor_tensor(at[:], av_saved[h][:], bcs[:], MUL)
                            attn_saved.append(at)
                        for i in range(4):
                            r0 = q0 + i * 128
                            for e in range(4):
                                fp = mfin.tile([128, 512], f32, tag="mf", name="fp")
                                for h in range(HPC):
                                    nc.tensor.matmul(fp[:], attn_saved[h][:, i * 128:(i + 1) * 128],
                                                     wo_sb[:, h, e * 512:(e + 1) * 512],
                                                     start=(h == 0), stop=(h == HPC - 1))
                                fo = fsb.tile([128, 512], f32, tag="fo")
                                nc.vector.tensor_copy(fo[:], fp[:])
                                nc.sync.dma_start(out[r0:r0 + 128, e * 512:(e + 1) * 512], fo[:])
                p2.close()

    nc.compile()
    return nc


def _get_exec():
    """Build the Bass module once and wrap it in a cached jitted shard_map
    executable (mirrors concourse.bass2jax.run_bass_via_pjrt, minus donation so
    repeated calls can reuse device-resident buffers)."""
    if "exec" in _cache:
        return _cache["exec"]
    import jax
    import concourse.mybir as mybir
    from jax.experimental.shard_map import shard_map
    from jax.sharding import Mesh, PartitionSpec
    from concourse import bass2jax

    nc = _build()
    bass2jax.install_neuronx_cc_hook()

    partition_name = nc.partition_id_tensor.name if nc.partition_id_tensor else None
    in_names, out_names, out_avals = [], [], []
    for alloc in nc.m.functions[0].allocations:
        if not isinstance(alloc, mybir.__dict__["MemoryLocationSet"]):
            continue
        name = alloc.memorylocations[0].name
        if alloc.kind == "ExternalInput":
            if name != partition_name:
                in_names.append(name)
        elif alloc.kind == "ExternalOutput":
            out_names.append(name)
            out_avals.append(jax.core.ShapedArray(tuple(alloc.tensor_shape),
                                                  mybir.dt.np(alloc.dtype)))
    n_params = len(in_names)
    in_names = in_names + out_names  # zero-buffer operands, per bass2jax contract
    if partition_name is not None:
        in_names.append(partition_name)

    def _body(*args):
        operands = list(args)
        if partition_name is not None:
            operands.append(bass2jax.partition_id_tensor())
        outs = bass2jax._bass_exec_p.bind(
            *operands,
            out_avals=tuple(out_avals),
            in_names=tuple(in_names),
            out_names=tuple(out_names),
            lowering_input_output_aliases=(),
            sim_require_finite=True,
            sim_require_nnan=True,
            nc=nc,
        )
        return tuple(outs)

    devices = jax.devices()[:NCORES]
    mesh = Mesh(np.asarray(devices), ("core",))
    spec = PartitionSpec("core")
    sharded = jax.jit(
        shard_map(_body, mesh=mesh,
                  in_specs=(spec,) * (n_params + len(out_names)),
                  out_specs=(spec,) * len(out_names),
                  check_rep=False),
        keep_unused=True,
    )
    _cache["exec"] = {
        "sharded": sharded, "in_names": in_names, "out_names": out_names,
        "out_avals": out_avals, "n_params": n_params, "mesh": mesh, "spec": spec,
    }
    return _cache["exec"]


def _prep_in_maps(x, wq, wk, wv, wo, key_weights):
    x = np.ascontiguousarray(np.asarray(x, dtype=np.float32))
    wq = np.asarray(wq, dtype=np.float32)
    wk = np.asarray(wk, dtype=np.float32)
    wv = np.asarray(wv, dtype=np.float32)
    wo = np.asarray(wo, dtype=np.float32)
    key_weights = np.asarray(key_weights, dtype=np.float32)

    xT = np.ascontiguousarray(x.reshape(SQ, HID).T)          # [HID, SQ]
    wqT = np.ascontiguousarray(wq.T)                         # [HID, NH*D]
    wkT = np.ascontiguousarray(wk.T)                         # [HID, NKV*D]
    wvT = np.ascontiguousarray(wv.T)
    woT = np.ascontiguousarray(wo.T)                         # [NH*D, HID]

    in_maps = []
    for c in range(NCORES):
        kv = c // 2
        in_maps.append({
            "xT": xT,
            "wqT": np.ascontiguousarray(wqT[:, c * HPC * D:(c + 1) * HPC * D]),
            "wkT": np.ascontiguousarray(wkT[:, kv * D:(kv + 1) * D]),
            "wvT": np.ascontiguousarray(wvT[:, kv * D:(kv + 1) * D]),
            "woT": np.ascontiguousarray(woT[c * HPC * D:(c + 1) * HPC * D, :]),
            "kw": np.ascontiguousarray(key_weights[c * HPC:(c + 1) * HPC]),
        })
    return in_maps


def _concat_args(ex, in_maps):
    concat_in = [
        np.concatenate([np.asarray(in_maps[c][name]) for c in range(NCORES)], axis=0)
        for name in ex["in_names"][:ex["n_params"]]
    ]
    zeros = [
        np.zeros((NCORES * av.shape[0], *av.shape[1:]), av.dtype)
        for av in ex["out_avals"]
    ]
    return concat_in + zeros


def kernel(x, wq, wk, wv, wo, key_weights):
    ex = _get_exec()
    in_maps = _prep_in_maps(x, wq, wk, wv, wo, key_weights)
    args = _concat_args(ex, in_maps)
    out_arrs = ex["sharded"](*args)
    total = np.asarray(out_arrs[0]).reshape(NCORES, SQ, HID).sum(axis=0, dtype=np.float32)
    return total.reshape(B, S, HID)



# revision 7
# speedup vs baseline: 175.3712x; 175.3712x over previous
"""GQA (grouped-query attention) Trainium2 kernel, tensor-parallel over 8 NeuronCores.

Sharding: core c computes query heads {2c, 2c+1} and kv head c//2 (groups kept
intact), with wo row-sharded; each core returns a partial [B*S, HID] output and
the host sums the 8 partials.

Per-core device kernel (matmuls in bf16, f32 PSUM accumulate; rel err ~5e-3):
  phase 1: Q^T/K^T/V^T projections from host-transposed bf16 x^T, RoPE on Q/K
           (DVE), V to natural layout via DMA-xbar transpose. Weights stream in
           per-ko chunks so the first matmul starts ~2us in.
  phase 2: per (batch, 512-wide q-block, head): scores^T = K_blk Q^T (PE) with
           the causal mask folded in as a -1e38 constant-add matmul on the
           diagonal blocks -> exp (ACT, key_weight*scale folded into the
           activation scale; assumes key_weights > 0) -> attn^T accum +
           sum-of-exp via ones-matmul (PE). Softmax normalization: per-head
           reciprocal_approx_fast (DVE) + partition_broadcast (GpSimd) +
           attn^T * bcast (DVE). wo projection consumes attn^T directly and is
           emitted one block late so its matmuls fill the next block's
           exp-latency bubbles; output DMAd straight from PSUM.
"""
import numpy as np
import ml_dtypes

BF16 = ml_dtypes.bfloat16

B, S, HID = 2, 2048, 2048
NH, NKV, D = 16, 4, 128
NCORES = 8
HPC = NH // NCORES            # q heads per core
SQ = B * S                    # 4096 tokens
NKO = HID // 128              # 16 contraction chunks
NSQB = SQ // 512              # 8 sequence blocks of 512
JPB = S // 512                # 4 q-blocks per batch
KPB = S // 128                # 16 k-blocks per batch
ROPE_BASE = 10000.0
SCALE = float(D) ** -0.5
NEG = -1.0e38

_cache = {}


def _consts():
    half = D // 2
    pos = np.arange(S, dtype=np.float32)
    inv_freq = (1.0 / (ROPE_BASE ** (np.arange(half, dtype=np.float32) / np.float32(half)))).astype(np.float32)
    ang = pos[:, None] * inv_freq[None, :]              # [S, 64]
    cos = np.cos(ang).astype(np.float32).T              # [64, S]
    sin = np.sin(ang).astype(np.float32).T
    cos_full = np.tile(np.concatenate([cos, cos], 0), (1, B)).astype(BF16)   # [128, SQ]
    sinpm = np.tile(np.concatenate([-sin, sin], 0), (1, B)).astype(BF16)     # [128, SQ]
    r = np.arange(128)
    # maskA[r, c] = 0 if c >= r else NEG   (within-block causal)
    maskA = np.where(r[None, :] >= r[:, None], 0.0, NEG).astype(BF16)        # [128,128]
    # maskB[r, c] = 0 if c - 128 - r >= 0 else NEG  (last diag block, 256 cols)
    c256 = np.arange(256)
    maskB = np.where(c256[None, :] - 128 - r[:, None] >= 0, 0.0, NEG).astype(BF16)  # [128,256]
    ident = np.eye(128, dtype=np.float32).astype(BF16)
    ones_col = np.ones((128, 1), np.float32).astype(BF16)
    return cos_full, sinpm, maskA, maskB, ident, ones_col


def _build(phases=(1, 2)):
    import concourse.mybir as mybir
    from concourse import bacc
    from concourse.tile import TileContext
    from contextlib import ExitStack

    f32 = mybir.dt.float32
    bf16 = mybir.dt.bfloat16
    MUL = mybir.AluOpType.mult
    ADD = mybir.AluOpType.add
    EXP = mybir.ActivationFunctionType.Exp
    CPY = mybir.ActivationFunctionType.Copy

    cos_np, sinpm_np, maskA_np, maskB_np, ident_np, onescol_np = _consts()

    nc = bacc.Bacc("TRN2", target_bir_lowering=False, debug=False)

    xT = nc.dram_tensor("xT", [HID, SQ], bf16, kind="ExternalInput")
    wqT = nc.dram_tensor("wqT", [HID, HPC * D], bf16, kind="ExternalInput")
    wkT = nc.dram_tensor("wkT", [HID, D], bf16, kind="ExternalInput")
    wvT = nc.dram_tensor("wvT", [HID, D], bf16, kind="ExternalInput")
    woT = nc.dram_tensor("woT", [HPC * D, HID], bf16, kind="ExternalInput")
    kw = nc.dram_tensor("kw", [HPC], f32, kind="ExternalInput")
    out = nc.dram_tensor("out", [SQ, HID], f32, kind="ExternalOutput")

    cos_d = nc.inline_tensor(cos_np, name="cos_t")
    sinpm_d = nc.inline_tensor(sinpm_np, name="sinpm_t")
    maskA_d = nc.inline_tensor(maskA_np, name="maskA_t")
    maskB_d = nc.inline_tensor(maskB_np, name="maskB_t")
    ident_d = nc.inline_tensor(ident_np, name="ident_t")
    onescol_d = nc.inline_tensor(onescol_np, name="onescol_t")

    with TileContext(nc) as tc:
        with tc.tile_pool(name="persist", bufs=1) as pp:
            # persistent SBUF tensors (all matmul operands bf16)
            wq_sb = pp.tile([128, NKO, HPC * D], bf16, tag="wq")
            wk_sb = pp.tile([128, NKO, D], bf16, tag="wk")
            wv_sb = pp.tile([128, NKO, D], bf16, tag="wv")
            wo_sb = pp.tile([128, HPC, HID], bf16, tag="wo")
            q_sb = [pp.tile([128, SQ], bf16, tag=f"q{h}", name=f"q_sb{h}") for h in range(HPC)]
            k_sb = pp.tile([128, SQ], bf16, tag="k")
            v_sb = pp.tile([128, SQ // 128, D], bf16, tag="v")
            cos_sb = pp.tile([128, SQ], bf16, tag="cos")
            sinpm_sb = pp.tile([128, SQ], bf16, tag="sinpm")
            maskA_sb = pp.tile([128, 128], bf16, tag="maskA")
            maskB_sb = pp.tile([128, 256], bf16, tag="maskB")
            ident_sb = pp.tile([128, 128], bf16, tag="ident")
            onescol_sb = pp.tile([128, 1], bf16, tag="onescol")
            kwsc_sb = pp.tile([128, HPC], f32, tag="kwsc")
            kwraw_sb = pp.tile([128, HPC], f32, tag="kwraw")

            es = ExitStack()
            rt = es.enter_context(tc.tile_pool(name="rt", bufs=2))
            vs = es.enter_context(tc.tile_pool(name="vs", bufs=2))
            ep = es.enter_context(tc.tile_pool(name="ep", bufs=3))
            ap = es.enter_context(tc.tile_pool(name="ap", bufs=4))
            bp = es.enter_context(tc.tile_pool(name="bp", bufs=2))
            smp = es.enter_context(tc.tile_pool(name="smp", bufs=4))
            fsb = es.enter_context(tc.tile_pool(name="fsb", bufs=4))

            def rope(dst, src_ps, s0):
                t0 = rt.tile([128, 512], f32, tag="t0")
                t1 = rt.tile([128, 512], f32, tag="t1")
                nc.vector.tensor_tensor(t0[:], src_ps[:], cos_sb[:, s0:s0 + 512], MUL)
                nc.vector.tensor_tensor(t1[0:64, :], src_ps[64:128, :], sinpm_sb[0:64, s0:s0 + 512], MUL)
                nc.vector.tensor_tensor(t1[64:128, :], src_ps[0:64, :], sinpm_sb[64:128, s0:s0 + 512], MUL)
                nc.vector.tensor_tensor(dst, t0[:], t1[:], ADD)

            def proj_block(pps, vpsp, xp, sb):
                s0 = sb * 512
                qps = [pps.tile([128, 512], f32, tag=f"qps{h}", name=f"qps{h}") for h in range(HPC)]
                kps = pps.tile([128, 512], f32, tag="kps", name="kps")
                vps = vpsp.tile([128, 512], f32, tag="vps", name="vps")
                for ko in range(NKO):
                    if sb == 0:
                        # stream weight chunks on the scalar HWDGE queue so the
                        # first matmul starts as soon as chunk 0 lands
                        nc.scalar.dma_start(wq_sb[:, ko, :], wqT[ko * 128:(ko + 1) * 128, :])
                        nc.scalar.dma_start(wk_sb[:, ko, :], wkT[ko * 128:(ko + 1) * 128, :])
                        nc.scalar.dma_start(wv_sb[:, ko, :], wvT[ko * 128:(ko + 1) * 128, :])
                    if sb == 0 and ko == 4:
                        # rope consts early enough that rope(sb0) doesn't gate
                        # the pps PSUM ring at sb2
                        nc.scalar.dma_start(cos_sb[:], cos_d[:])
                        nc.scalar.dma_start(sinpm_sb[:], sinpm_d[:])
                        nc.scalar.dma_start(kwraw_sb[:], kw[None, :].to_broadcast((128, HPC)))
                        nc.vector.tensor_scalar_mul(kwsc_sb[:], kwraw_sb[:], SCALE)
                    xt = xp.tile([128, 512], bf16, tag="x")
                    nc.sync.dma_start(xt[:], xT[ko * 128:(ko + 1) * 128, s0:s0 + 512])
                    st, sp = (ko == 0), (ko == NKO - 1)
                    for h in range(HPC):
                        nc.tensor.matmul(qps[h][:], wq_sb[:, ko, h * D:(h + 1) * D], xt[:], start=st, stop=sp)
                    nc.tensor.matmul(kps[:], wk_sb[:, ko, :], xt[:], start=st, stop=sp)
                    nc.tensor.matmul(vps[:], wv_sb[:, ko, :], xt[:], start=st, stop=sp)
                if sb == 1:
                    nc.scalar.dma_start(wo_sb[:], woT[:].rearrange("(h p) e -> p h e", p=128))
                    nc.scalar.dma_start(maskA_sb[:], maskA_d[:])
                    nc.scalar.dma_start(maskB_sb[:], maskB_d[:])
                    nc.scalar.dma_start(ident_sb[:], ident_d[:])
                    nc.scalar.dma_start(onescol_sb[:], onescol_d[:])
                # V staging copy (ACT) so the vps bank frees fast, then
                # DMA-xbar transpose [d, s] -> natural [s, d]
                vst = vs.tile([128, 512], bf16, tag="vst")
                nc.scalar.activation(vst[:], vps[:], CPY)
                for h in range(HPC):
                    rope(q_sb[h][:, s0:s0 + 512], qps[h], s0)
                rope(k_sb[:, s0:s0 + 512], kps, s0)
                for i in range(4):
                    nc.scalar.dma_start_transpose(v_sb[:, sb * 4 + i, :], vst[:, i * 128:(i + 1) * 128])

            if 1 not in phases:  # ablation-timing only: fabricate phase-1 outputs
                for h in range(HPC):
                    nc.gpsimd.memset(q_sb[h][:], 0.5)
                nc.gpsimd.memset(k_sb[:], 0.5)
                nc.gpsimd.memset(v_sb[:], 0.5)

            OFF = [0, 128, 256, 256]
            if 1 in phases:
                with tc.tile_pool(name="pps", bufs=2, space="PSUM") as pps, \
                     tc.tile_pool(name="vps_p", bufs=1, space="PSUM") as vpsp, \
                     tc.tile_pool(name="xp", bufs=6) as xp:
                    for sb in range(NSQB):
                        proj_block(pps, vpsp, xp, sb)

            p2 = ExitStack()
            if 2 in phases:
                scps = p2.enter_context(tc.tile_pool(name="scps", bufs=2, space="PSUM"))
                avps = p2.enter_context(tc.tile_pool(name="avps", bufs=2, space="PSUM"))
                sups = p2.enter_context(tc.tile_pool(name="sups", bufs=1, space="PSUM"))
                mfin = p2.enter_context(tc.tile_pool(name="mfin", bufs=3, space="PSUM"))

            # deferred wo work from the previous (b, J) block: emitting it
            # between the next block's score/exp matmuls keeps the PE warm
            # through the softmax finish
            pending = []
            fo_engines = [nc.vector, nc.scalar, nc.vector]  # gpsimd can't read PSUM
            fo_rr = [0]

            def emit_wo(n):
                for _ in range(n):
                    if not pending:
                        return
                    q0w, ats, i, e = pending.pop(0)
                    fp = mfin.tile([128, 512], f32, tag="mf", name="fp")
                    for h in range(HPC):
                        nc.tensor.matmul(fp[:], ats[h][:, i * 128:(i + 1) * 128],
                                         wo_sb[:, h, e * 512:(e + 1) * 512],
                                         start=(h == 0), stop=(h == HPC - 1))
                    fo = fsb.tile([128, 512], f32, tag="fo")
                    eng = fo_engines[fo_rr[0] % 3]
                    fo_rr[0] += 1
                    if eng is nc.scalar:
                        eng.activation(fo[:], fp[:], CPY)
                    else:
                        eng.tensor_copy(fo[:], fp[:])
                    nc.sync.dma_start(out[q0w + i * 128:q0w + (i + 1) * 128,
                                          e * 512:(e + 1) * 512], fo[:])

            for b in (range(B) if 2 in phases else ()):
                t0 = b * S
                for J in range(JPB):
                    q0 = t0 + J * 512
                    nkb = 4 * J + 4
                    sup = sups.tile([33, 512], f32, tag="su")
                    attn_now = []
                    for h in range(HPC):
                        avp = avps.tile([128, 512], f32, tag="av", name="avp")
                        for jj in range(nkb):
                            p = jj - 4 * J
                            off = OFF[p] if p >= 0 else 0
                            n = 512 - off
                            scp = scps.tile([128, 512], f32, tag="sc", name="scp")
                            diag = (p >= 0)
                            nc.tensor.matmul(
                                scp[:, 0:n],
                                k_sb[:, t0 + jj * 128:t0 + (jj + 1) * 128],
                                q_sb[h][:, q0 + off:q0 + 512],
                                start=True, stop=not diag)
                            if diag:
                                # causal mask: add -1e38 to the sub-diagonal of
                                # the block (valid for key_weights > 0)
                                if p == 3:
                                    nc.tensor.matmul(scp[:, 0:256], ident_sb[:],
                                                     maskB_sb[:], start=False, stop=True)
                                else:
                                    nc.tensor.matmul(scp[:, 0:128], ident_sb[:],
                                                     maskA_sb[:], start=False, stop=True)
                            ex = ep.tile([128, 512], bf16, tag="ex")
                            nc.scalar.activation(ex[:, 0:n], scp[:, 0:n], EXP,
                                                 scale=kwsc_sb[:, h:h + 1])
                            st, sp = (jj == 0), (jj == nkb - 1)
                            nc.tensor.matmul(avp[:, off:512], v_sb[:, b * KPB + jj, :],
                                             ex[:, 0:n], start=st, stop=sp)
                            nc.tensor.matmul(sup[32 * h:32 * h + 1, off:512], onescol_sb[:],
                                             ex[:, 0:n], start=st, stop=sp)
                        # normalization chain runs on DVE/GpSimd under the other
                        # head's (or deferred wo) PE work
                        recip = smp.tile([1, 512], f32, tag=f"recip{h}", name="recip")
                        nc.vector.reciprocal_approx_fast(recip[:], sup[32 * h:32 * h + 1, :])
                        bcs = bp.tile([128, 512], f32, tag="bcs")
                        nc.gpsimd.partition_broadcast(bcs[:], recip[:])
                        at = ap.tile([128, 512], bf16, tag="at")
                        nc.vector.tensor_tensor(at[:], avp[:], bcs[:], MUL)
                        attn_now.append(at)
                        emit_wo(8)
                    emit_wo(len(pending))  # drain any leftovers (J=0 blocks)
                    ats = list(attn_now)
                    for i in range(4):
                        for e in range(4):
                            pending.append((q0, ats, i, e))
            emit_wo(len(pending))
            p2.close()
            es.close()

    nc.compile()
    return nc


def _get_exec():
    """Build the Bass module once and wrap it in a cached jitted shard_map
    executable (mirrors concourse.bass2jax.run_bass_via_pjrt, minus donation so
    repeated calls can reuse device-resident buffers)."""
    if "exec" in _cache:
        return _cache["exec"]
    import jax
    import concourse.mybir as mybir
    from jax.experimental.shard_map import shard_map
    from jax.sharding import Mesh, PartitionSpec
    from concourse import bass2jax

    nc = _build()
    bass2jax.install_neuronx_cc_hook()

    partition_name = nc.partition_id_tensor.name if nc.partition_id_tensor else None
    in_names, out_names, out_avals = [], [], []
    for alloc in nc.m.functions[0].allocations:
        if not isinstance(alloc, mybir.__dict__["MemoryLocationSet"]):
            continue
        name = alloc.memorylocations[0].name
        if alloc.kind == "ExternalInput":
            if name != partition_name:
                in_names.append(name)
        elif alloc.kind == "ExternalOutput":
            out_names.append(name)
            out_avals.append(jax.core.ShapedArray(tuple(alloc.tensor_shape),
                                                  mybir.dt.np(alloc.dtype)))
    n_params = len(in_names)
    in_names = in_names + out_names  # zero-buffer operands, per bass2jax contract
    if partition_name is not None:
        in_names.append(partition_name)

    def _body(*args):
        operands = list(args)
        if partition_name is not None:
            operands.append(bass2jax.partition_id_tensor())
        outs = bass2jax._bass_exec_p.bind(
            *operands,
            out_avals=tuple(out_avals),
            in_names=tuple(in_names),
            out_names=tuple(out_names),
            lowering_input_output_aliases=(),
            sim_require_finite=True,
            sim_require_nnan=True,
            nc=nc,
        )
        return tuple(outs)

    devices = jax.devices()[:NCORES]
    mesh = Mesh(np.asarray(devices), ("core",))
    spec = PartitionSpec("core")
    sharded = jax.jit(
        shard_map(_body, mesh=mesh,
                  in_specs=(spec,) * (n_params + len(out_names)),
                  out_specs=(spec,) * len(out_names),
                  check_rep=False),
        keep_unused=True,
    )
    _cache["nc"] = nc
    _cache["exec"] = {
        "sharded": sharded, "in_names": in_names, "out_names": out_names,
        "out_avals": out_avals, "n_params": n_params, "mesh": mesh, "spec": spec,
    }
    return _cache["exec"]


def _prep_in_maps(x, wq, wk, wv, wo, key_weights):
    x = np.ascontiguousarray(np.asarray(x, dtype=np.float32))
    wq = np.asarray(wq, dtype=np.float32)
    wk = np.asarray(wk, dtype=np.float32)
    wv = np.asarray(wv, dtype=np.float32)
    wo = np.asarray(wo, dtype=np.float32)
    key_weights = np.asarray(key_weights, dtype=np.float32)

    xT = np.ascontiguousarray(x.reshape(SQ, HID).T.astype(BF16))   # [HID, SQ]
    wqT = np.ascontiguousarray(wq.T.astype(BF16))                  # [HID, NH*D]
    wkT = np.ascontiguousarray(wk.T.astype(BF16))                  # [HID, NKV*D]
    wvT = np.ascontiguousarray(wv.T.astype(BF16))
    woT = np.ascontiguousarray(wo.T.astype(BF16))                  # [NH*D, HID]

    in_maps = []
    for c in range(NCORES):
        kv = c // 2
        in_maps.append({
            "xT": xT,
            "wqT": np.ascontiguousarray(wqT[:, c * HPC * D:(c + 1) * HPC * D]),
            "wkT": np.ascontiguousarray(wkT[:, kv * D:(kv + 1) * D]),
            "wvT": np.ascontiguousarray(wvT[:, kv * D:(kv + 1) * D]),
            "woT": np.ascontiguousarray(woT[c * HPC * D:(c + 1) * HPC * D, :]),
            "kw": np.ascontiguousarray(key_weights[c * HPC:(c + 1) * HPC]),
        })
    return in_maps


def _concat_args(ex, in_maps):
    concat_in = [
        np.concatenate([np.asarray(in_maps[c][name]) for c in range(NCORES)], axis=0)
        for name in ex["in_names"][:ex["n_params"]]
    ]
    zeros = [
        np.zeros((NCORES * av.shape[0], *av.shape[1:]), av.dtype)
        for av in ex["out_avals"]
    ]
    return concat_in + zeros


def kernel(x, wq, wk, wv, wo, key_weights):
    ex = _get_exec()
    in_maps = _prep_in_maps(x, wq, wk, wv, wo, key_weights)
    args = _concat_args(ex, in_maps)
    out_arrs = ex["sharded"](*args)
    total = np.asarray(out_arrs[0]).reshape(NCORES, SQ, HID).sum(axis=0, dtype=np.float32)
    return total.reshape(B, S, HID)
